# revision 23
# baseline (speedup 1.0000x reference)
"""Deformable Conv2d (K=3, stride 1, pad 1, dil 1) on 8 TRN2 NeuronCores.

Sharding: data-parallel over (batch=4) x (H halves=2) -> 8 cores.
Each core computes out[b, :, h0:h0+64, :] for its (b, h0).

v3 pipeline (dma_gather + bf16):
  1. offset conv (18ch) via PE f32 matmuls over a 1px-zero-padded image.
  2. PE-transpose offsets to point-major [128pts, 18].
  3. DVE coord math (f32): ys/xs, magic-number floor, frac (stored bf16),
     bilinear corner weights w00..w11 (bf16), int16 gather row indices
     into a 2px-zero-padded channels-last 4C-packed bf16 image in DRAM.
  4. idx shuffle to the SWDGE dma_gather wrapped layout (idx j read at
     partition 16 + j%16, halfword j//16 -- HW-probed): hop1 = 8 small
     SBUF->SBUF DMAs moving partition blocks, hop2 = 3 DVE int16 copies
     permuting free dims.
  5. Gather: 3x dma_gather (InstDMAGatherAnt, mlp gpsimd library) per
     2-group tile, NI=768 rows x 512B (2x2 px x 64ch bf16, 4C-packed).
     One Pool instruction generates 768 descriptors (vs 6 instructions
     of 128 each for indirect_dma_start); ucode ring caps NI at 1024.
  6. DVE bilinear lerp in bf16: s = w00*v00+w01*v01+w10*v10+w11*v11
     (7 wide ops per group).
  7. PE transpose S (bf16) to channel-major, main conv matmuls
     (K=576 as 4x128+64 accumulation, bf16), ACT bias add (f32), DMA out.
"""

import sys
for p in ("/opt/trn_rl_repo",):
    if p not in sys.path:
        sys.path.insert(0, p)

import numpy as np
import ml_dtypes

_BF = ml_dtypes.bfloat16

import concourse.bacc as bacc
import concourse.mybir as mybir
import concourse.tile as tile
import concourse.bass as bass
from concourse.bass_utils import run_bass_kernel_spmd
from concourse.library_config import mlp as _mlp_lib

F32 = mybir.dt.float32
F32R = mybir.dt.float32r
BF16 = mybir.dt.bfloat16
I16 = mybir.dt.int16
AL = mybir.AluOpType
AF = mybir.ActivationFunctionType

B, C, H, W = 4, 64, 128, 128
K, KK = 3, 9
O = 64                      # output channels
OC = 2 * KK                 # offset channels (18)
HL = H // 2                 # local rows per core (64)
NPT = HL * W                # local points per core (8192)
NG = NPT // 128             # point groups of 128 (=64); group g == local row g
W2 = W + 2                  # 1px-padded width for offset conv (130)
H2 = HL + 2                 # 1px-padded local rows (66)
W4 = W + 4                  # 2px-padded width for gather image (132)
H4 = H + 4                  # 2px-padded height (full image!) (132)
MAGIC = float(3 * 2 ** 22)   # 1.5*2^23: ulp stays 1.0 for f32 in [-2^22, 2^22]
GCH = 2                     # point-groups per gather tile
E = 256                     # gathered elems per (point, tap): 2x2 px x 64ch
GBLK = KK * E               # gathered elems per point per group (2304)
NB = GCH * KK               # blocks per gather tile (18)
NCALL = 3                   # dma_gather calls per tile (6 blocks each)
NI = (NB // NCALL) * 128    # idxs per dma_gather call (768; ucode cap 1024)
CHUNKS = 8                  # preamble chunks (groups per chunk = NG/CHUNKS)
CG = NG // CHUNKS           # groups per chunk (16)
NT = CG // GCH              # gather tiles per chunk (8)
WCOL = CG * KK * 8          # wrapped idx cols per chunk (1152)


def build_program(dbg=False, skip_gather=False, skip_lerp=False,
                  skip_mm=False, skip_off=False, reps=1):
    nc = bacc.Bacc("TRN2", target_bir_lowering=False, debug=False)

    xp = nc.dram_tensor("xp", [C, H2 * W2], BF16, kind="ExternalInput")
    xcl = nc.dram_tensor("xcl", [H4 * W4, 4 * C], BF16, kind="ExternalInput")
    wofft = nc.dram_tensor("wofft", [C, KK * OC], BF16, kind="ExternalInput")
    woffb = nc.dram_tensor("woffb", [OC, 1], F32, kind="ExternalInput")
    wmain = nc.dram_tensor("wmain", [128, 5 * O], BF16, kind="ExternalInput")
    wb = nc.dram_tensor("wb", [O, 1], F32, kind="ExternalInput")
    baseyx = nc.dram_tensor("baseyx", [128, NG * KK * 2], F32,
                            kind="ExternalInput")
    identb = nc.dram_tensor("identb", [128, 128], BF16, kind="ExternalInput")
    selr = nc.dram_tensor("selr", [128, 8 * 32], F32, kind="ExternalInput")
    out = nc.dram_tensor("out", [O, NPT], F32, kind="ExternalOutput")

    # before TileContext: guaranteed to precede every gather (the tile
    # scheduler is free to move dep-less instructions otherwise, and a
    # GPSIMD iram reload between inflight dma_gathers is fatal).
    nc.gpsimd.load_library(_mlp_lib)

    with tile.TileContext(nc) as tc:
        with (
            tc.tile_pool(name="cst", bufs=1) as cst,
            tc.tile_pool(name="psA", bufs=1, space="PSUM") as psA,
            tc.tile_pool(name="psT", bufs=3, space="PSUM") as psT,
            tc.tile_pool(name="psO", bufs=3, space="PSUM") as psO,
        ):
            # ---- load constants / weights (once) ----
            identb_t = cst.tile([128, 128], BF16, tag="identb")
            nc.sync.dma_start(out=identb_t[:], in_=identb[:])
            selr_t = cst.tile([128, 8 * 32], F32, tag="selr")
            nc.sync.dma_start(out=selr_t[:], in_=selr[:])
            wofft_t = cst.tile([C, KK * OC], BF16, tag="wofft")
            nc.sync.dma_start(out=wofft_t[:], in_=wofft[:])
            woffb_t = cst.tile([OC, 1], F32, tag="woffb")
            nc.sync.dma_start(out=woffb_t[:], in_=woffb[:])
            wmain_t = cst.tile([128, 5 * O], BF16, tag="wmain")
            nc.sync.dma_start(out=wmain_t[:], in_=wmain[:])
            wb_t = cst.tile([O, 1], F32, tag="wb")
            nc.sync.dma_start(out=wb_t[:], in_=wb[:])
            baseyx_t = cst.tile([128, NG * KK * 2], F32, tag="baseyx")
            nc.sync.dma_start(out=baseyx_t[:], in_=baseyx[:])

            for rep in range(reps):
                build_body(nc, tc, psA, psT, psO, rep,
                           selr_t, identb_t, wofft_t, woffb_t, wmain_t,
                           wb_t, baseyx_t, xp, xcl, out,
                           skip_gather, skip_lerp, skip_mm, skip_off)

    nc.compile()
    return nc


def build_body(nc, tc, psA, psT, psO, rep,
               selr_t, identb_t, wofft_t, woffb_t, wmain_t, wb_t,
               baseyx_t, xp, xcl, out,
               skip_gather, skip_lerp, skip_mm, skip_off):
    with (
        tc.tile_pool(name=f"early{rep}", bufs=1) as early,
        tc.tile_pool(name=f"coord{rep}", bufs=1) as coord,
        tc.tile_pool(name=f"ctmp{rep}", bufs=2) as ctmp,
        tc.tile_pool(name=f"gat{rep}", bufs=4) as gat,
        tc.tile_pool(name=f"lrp{rep}", bufs=2) as lrp,
        tc.tile_pool(name=f"outp{rep}", bufs=1) as outp,
    ):
        xp_t = early.tile([C, H2 * W2], BF16, tag="xp")
        nc.sync.dma_start(out=xp_t[:], in_=xp[:])
        xp3 = xp_t[:].rearrange("c (h w) -> c h w", h=H2)

        # per-chunk coord tiles so chunk-0 gathers don't wait on chunk-3
        w_c = []      # per chunk: [w00, w01, w10, w11] bf16 [128, CG*KK]
        wrap_c = []   # per chunk: wrapped int16 idx [128, WCOL]
        for ch in range(CHUNKS):
            w_c.append([coord.tile([128, CG * KK], BF16, tag=f"w{q}{ch}",
                                   name=f"w{q}{ch}_{rep}") for q in range(4)])
            wrap_c.append(coord.tile([128, WCOL], I16, tag=f"wr{ch}",
                                     name=f"wr{ch}_{rep}"))
        off_c = [coord.tile([OC, CG * W], BF16, tag=f"off{ch}",
                            name=f"off{ch}_{rep}")
                 for ch in range(CHUNKS)]
        out_sb = outp.tile([O, NPT], F32, tag="osb")
        if skip_mm:
            nc.vector.memset(out_sb[:], 0.0)

        def preamble_chunk(ch):
            g0 = ch * CG          # first group (= local row) of chunk
            off_t = off_c[ch]
            if skip_off:
                nc.vector.memset(off_t[:], 0.0)
            # ---- offset conv rows [g0, g0+CG) ----
            RPC = 4               # rows per psum chunk (N=512)
            for r0 in ([] if skip_off else range(g0, g0 + CG, RPC)):
                ps = psA.tile([OC, RPC * W], F32, tag="psA")
                for kk in range(KK):
                    ki, kj = kk // K, kk % K
                    rhs = xp3[:, r0 + ki:r0 + ki + RPC, kj:kj + W]
                    nc.tensor.matmul(
                        out=ps[:],
                        lhsT=wofft_t[:, kk * OC:(kk + 1) * OC],
                        rhs=rhs,
                        start=(kk == 0), stop=(kk == KK - 1))
                nc.scalar.activation(
                    out=off_t[:, (r0 - g0) * W:(r0 - g0 + RPC) * W], in_=ps[:],
                    func=AF.Identity, bias=woffb_t[:, 0:1], scale=1.0)

            # ---- transpose offsets to point-major [128, CG*18] ----
            offT = ctmp.tile([128, CG * OC], BF16, tag="offT")
            for gl in range(CG):
                ps = psT.tile([128, OC], BF16, tag="psTb")
                nc.tensor.transpose(
                    out=ps[:], in_=off_t[:, gl * 128:(gl + 1) * 128],
                    identity=identb_t[:OC, :OC])
                nc.scalar.copy(out=offT[:, gl * OC:(gl + 1) * OC], in_=ps[:])

            # ---- coordinate math on the (g,k,(y|x))-interleaved layout
            # offT is already (dy,dx)-interleaved, so ys|xs, floor, frac,
            # 1-frac and clamp all run as single double-width ops.
            NW = CG * KK
            yx = ctmp.tile([128, 2 * NW], F32, tag="yx")
            rr = ctmp.tile([128, 2 * NW], F32, tag="rr")
            mm_ = ctmp.tile([128, 2 * NW], F32, tag="mm")
            yx0 = ctmp.tile([128, 2 * NW], F32, tag="yx0")
            fyx = ctmp.tile([128, 2 * NW], BF16, tag="fyx")
            gyx = ctmp.tile([128, 2 * NW], BF16, tag="gyx")
            ti = ctmp.tile([128, NW], F32, tag="ti")
            idxf = ctmp.tile([128, NW], F32, tag="idxf")

            nc.vector.tensor_tensor(
                out=yx[:], in0=offT[:],
                in1=baseyx_t[:, g0 * 2 * KK:(g0 + CG) * 2 * KK], op=AL.add)
            # magic-number round-to-nearest, then fix round-ups
            nc.vector.tensor_scalar(
                out=rr[:], in0=yx[:], scalar1=MAGIC, scalar2=MAGIC,
                op0=AL.add, op1=AL.subtract)
            nc.vector.tensor_tensor(out=mm_[:], in0=rr[:], in1=yx[:],
                                    op=AL.is_gt)
            nc.vector.tensor_tensor(out=yx0[:], in0=rr[:], in1=mm_[:],
                                    op=AL.subtract)
            nc.vector.tensor_tensor(out=fyx[:], in0=yx[:], in1=yx0[:],
                                    op=AL.subtract)
            # gyx = (fyx - 1) * -1 = 1 - fyx
            nc.vector.tensor_scalar(out=gyx[:], in0=fyx[:], scalar1=1.0,
                                    scalar2=-1.0, op0=AL.subtract,
                                    op1=AL.mult)
            # corner weights from interleaved frac views
            f4 = fyx[:].rearrange("p (g k t) -> p g k t", g=CG, k=KK)
            g4 = gyx[:].rearrange("p (g k t) -> p g k t", g=CG, k=KK)
            fy_v, fx_v = f4[:, :, :, 0], f4[:, :, :, 1]
            gy_v, gx_v = g4[:, :, :, 0], g4[:, :, :, 1]
            w00, w01, w10, w11 = w_c[ch]
            w003 = w00[:].rearrange("p (g k) -> p g k", g=CG)
            w013 = w01[:].rearrange("p (g k) -> p g k", g=CG)
            w103 = w10[:].rearrange("p (g k) -> p g k", g=CG)
            w113 = w11[:].rearrange("p (g k) -> p g k", g=CG)
            nc.vector.tensor_tensor(out=w003, in0=gy_v, in1=gx_v, op=AL.mult)
            nc.vector.tensor_tensor(out=w013, in0=gy_v, in1=fx_v, op=AL.mult)
            nc.vector.tensor_tensor(out=w103, in0=fy_v, in1=gx_v, op=AL.mult)
            nc.vector.tensor_tensor(out=w113, in0=fy_v, in1=fx_v, op=AL.mult)
            # clamp both coords (same bounds: [-2, 128] since H == W)
            nc.vector.tensor_scalar(out=rr[:], in0=yx0[:], scalar1=-2.0,
                                    scalar2=float(H), op0=AL.max, op1=AL.min)
            # idx = (y0c*W4 + x0c) + (2*W4+2), f32 (exact integers)
            r4 = rr[:].rearrange("p (g k t) -> p g k t", g=CG, k=KK)
            ti3 = ti[:].rearrange("p (g k) -> p g k", g=CG)
            nc.vector.scalar_tensor_tensor(
                out=ti3, in0=r4[:, :, :, 0], scalar=float(W4),
                in1=r4[:, :, :, 1], op0=AL.mult, op1=AL.add)
            nc.vector.tensor_scalar(
                out=idxf[:], in0=ti[:], scalar1=float(2 * W4 + 2),
                scalar2=None, op0=AL.add)

            # ---- shuffle idxf [128 pts, NW] -> SWDGE wrapped layout ----
            # Gather ucode reads idx j at (partition 16 + j%16,
            # halfword j//16) of the idxs AP (HW-probed).  Target cell for
            # idx j of (tile t, call c, block b, point p=16*r+q):
            # (16+q, t*144 + c*48 + b*8 + r).  All-compute path (PE fold +
            # DVE strided int16 write) so the gather depends only on
            # engine semaphores, not rotating DMA-completion counters.
            ovw = wrap_c[ch][0:32, :].rearrange(
                "p (t c b r) -> p t c b r", t=NT, c=NCALL, b=NB // NCALL)
            for r in range(8):
                # psW[16+q, col] = idxf[16*r+q, col]; rows 0..15 zero
                ps = psA.tile([32, NW], F32, tag="psW")
                nc.tensor.matmul(
                    out=ps[:], lhsT=selr_t[:, r * 32:(r + 1) * 32],
                    rhs=idxf[:], start=True, stop=True)
                # f32 -> int16 with the (t,c,b) -> stride-8 col scatter
                nc.vector.tensor_scalar(
                    out=ovw[:, :, :, :, r], in0=ps[:].rearrange(
                        "p (t c b) -> p t c b", t=NT, c=NCALL),
                    scalar1=0, scalar2=None, op0=AL.add)

        def main_groups(ch):
            g0 = ch * CG
            for c0 in range(g0, g0 + CG, GCH):
                t = (c0 - g0) // GCH      # tile index within chunk
                gt = gat.tile([128, GCH * GBLK], BF16, tag="G")
                if skip_gather and not skip_lerp:
                    nc.vector.memset(gt[:], 0.0)
                # 3 dma_gather calls of 6 blocks (768 idxs) each: one Pool
                # instruction generates 768 descriptors of 512B.  NOTE:
                # multi-queue round-robin (num_swdge_queues=2) hangs the
                # mesh in this kernel -- single queue only.
                for c in ([] if skip_gather else range(NCALL)):
                    nc.gpsimd.dma_gather(
                        out_ap=gt[:, c * 6 * E:(c + 1) * 6 * E].rearrange(
                            "p (g e) -> p g e", e=E),
                        in_ap=xcl[:],
                        idxs_ap=wrap_c[ch][:, t * 144 + c * 48:
                                           t * 144 + (c + 1) * 48],
                        num_idxs=NI, num_idxs_reg=NI, elem_size=E,
                        queue_num=0)
                for gs in range(GCH):
                    g = c0 + gs
                    gl = g - g0
                    g5 = gt[:, gs * GBLK:(gs + 1) * GBLK].rearrange(
                        "p (k r q c) -> p k r q c", k=KK, r=2, q=2)
                    v00 = g5[:, :, 0, 0, :]
                    v01 = g5[:, :, 0, 1, :]
                    v10 = g5[:, :, 1, 0, :]
                    v11 = g5[:, :, 1, 1, :]

                    def wb_(wt):
                        return wt[:, gl * KK:(gl + 1) * KK].unsqueeze(2) \
                            .to_broadcast([128, KK, C])

                    d_ = lrp.tile([128, KK * C], BF16, tag="d")
                    m_ = lrp.tile([128, KK * C], BF16, tag="m")
                    l0 = lrp.tile([128, KK * C], BF16, tag="l0")
                    l1 = lrp.tile([128, KK * C], BF16, tag="l1")
                    sg = lrp.tile([128, KK * C], BF16, tag="s")
                    if skip_lerp and not skip_mm:
                        nc.vector.memset(sg[:], 0.0)
                    d3 = d_[:].rearrange("p (k c) -> p k c", k=KK)
                    m3 = m_[:].rearrange("p (k c) -> p k c", k=KK)
                    l03 = l0[:].rearrange("p (k c) -> p k c", k=KK)
                    l13 = l1[:].rearrange("p (k c) -> p k c", k=KK)
                    s3 = sg[:].rearrange("p (k c) -> p k c", k=KK)

                    if not skip_lerp:
                        w00, w01, w10, w11 = w_c[ch]
                        nc.vector.tensor_tensor(out=d3, in0=v00,
                                                in1=wb_(w00), op=AL.mult)
                        nc.vector.tensor_tensor(out=m3, in0=v01,
                                                in1=wb_(w01), op=AL.mult)
                        nc.vector.tensor_tensor(out=l03, in0=d3, in1=m3,
                                                op=AL.add)
                        nc.vector.tensor_tensor(out=d3, in0=v10,
                                                in1=wb_(w10), op=AL.mult)
                        nc.vector.tensor_tensor(out=m3, in0=v11,
                                                in1=wb_(w11), op=AL.mult)
                        nc.vector.tensor_tensor(out=l13, in0=d3, in1=m3,
                                                op=AL.add)
                        nc.vector.tensor_tensor(out=s3, in0=l03, in1=l13,
                                                op=AL.add)

                    if skip_mm:
                        continue
                    st = lrp.tile([128, 640], BF16, tag="st")
                    for j in range(4):
                        ps = psT.tile([128, 128], BF16, tag="psTb")
                        nc.tensor.transpose(
                            out=ps[:], in_=sg[:, j * 128:(j + 1) * 128],
                            identity=identb_t[:])
                        if j % 2 == 0:
                            nc.scalar.copy(out=st[:, j * 128:(j + 1) * 128],
                                           in_=ps[:])
                        else:
                            # balance PSUM->SBUF copies across ACT and DVE
                            nc.vector.tensor_scalar(
                                out=st[:, j * 128:(j + 1) * 128], in0=ps[:],
                                scalar1=0.0, scalar2=None, op0=AL.add)
                    ps = psT.tile([64, 128], BF16, tag="psTb")
                    nc.tensor.transpose(
                        out=ps[:], in_=sg[:, 512:576], identity=identb_t[:])
                    nc.scalar.copy(out=st[:64, 512:640], in_=ps[:])

                    po = psO.tile([O, 128], F32, tag="psO")
                    for j in range(4):
                        nc.tensor.matmul(
                            out=po[:], lhsT=wmain_t[:, j * O:(j + 1) * O],
                            rhs=st[:, j * 128:(j + 1) * 128],
                            start=(j == 0), stop=False)
                    nc.tensor.matmul(
                        out=po[:], lhsT=wmain_t[:64, 4 * O:5 * O],
                        rhs=st[:64, 512:640], start=False, stop=True)
                    nc.scalar.activation(
                        out=out_sb[:, g * 128:(g + 1) * 128], in_=po[:],
                        func=AF.Identity, bias=wb_t[:, 0:1], scale=1.0)
            # out DMA per chunk (overlaps with next chunk's work)
            nc.sync.dma_start(
                out=out[:, g0 * 128:(g0 + CG) * 128],
                in_=out_sb[:, g0 * 128:(g0 + CG) * 128])

        # chunk 0 preamble, then stream: gathers(ch) while preamble(ch+1)
        preamble_chunk(0)
        for ch in range(CHUNKS):
            if ch + 1 < CHUNKS:
                preamble_chunk(ch + 1)
            main_groups(ch)


_NC_CACHE = None


def _get_nc():
    global _NC_CACHE
    if _NC_CACHE is None:
        _NC_CACHE = build_program()
    return _NC_CACHE


def make_core_inputs(x, weight, bias, offset_w, offset_b):
    """Host-side prep: returns list of 8 in_maps (core i = batch i//2, half i%2)."""
    x = np.asarray(x, np.float32)
    weight = np.asarray(weight, np.float32)
    bias = np.asarray(bias, np.float32)
    offset_w = np.asarray(offset_w, np.float32)
    offset_b = np.asarray(offset_b, np.float32)

    xp_full = np.pad(x, ((0, 0), (0, 0), (1, 1), (1, 1)))
    xpad = np.pad(x, ((0, 0), (0, 0), (2, 2), (2, 3)))  # extra right col for x+1
    xpad = np.pad(xpad, ((0, 0), (0, 0), (0, 1), (0, 0)))  # extra bottom row
    xcl0 = xpad.transpose(0, 2, 3, 1)           # [B, 133, 133, C]
    zz = np.empty((B, H4, W4, 4 * C), np.float32)
    zz[..., 0 * C:1 * C] = xcl0[:, :H4, :W4, :]
    zz[..., 1 * C:2 * C] = xcl0[:, :H4, 1:W4 + 1, :]
    zz[..., 2 * C:3 * C] = xcl0[:, 1:H4 + 1, :W4, :]
    zz[..., 3 * C:4 * C] = xcl0[:, 1:H4 + 1, 1:W4 + 1, :]
    xcl_full = zz.astype(_BF)

    # offset conv weights: [c, kk*18], lhsT per tap
    wofft = np.ascontiguousarray(
        offset_w.reshape(OC, C, KK).transpose(1, 2, 0)).reshape(
            C, KK * OC).astype(_BF)
    woffb = offset_b.reshape(OC, 1)
    # main conv weights: [128, 5*64]; block j rows (t2*64+c), cols o
    wr = weight.reshape(O, C, KK)
    wmain = np.zeros((128, 5 * O), np.float32)
    for j in range(5):
        for t2 in range(2):
            kk = 2 * j + t2
            if kk >= KK:
                break
            wmain[t2 * C:(t2 + 1) * C, j * O:(j + 1) * O] = wr[:, :, kk].T
    wmain = wmain.astype(_BF)
    wb = bias.reshape(O, 1)
    identb = np.eye(128, dtype=np.float32).astype(_BF)
    # selr[p, r*32 + c] = 1 iff c >= 16 and p == 16*r + (c - 16):
    # fold matmul lhsT placing point 16*r+q at psum row 16+q
    selr = np.zeros((128, 8 * 32), np.float32)
    for r in range(8):
        for q in range(16):
            selr[16 * r + q, r * 32 + 16 + q] = 1.0

    p = np.arange(128, dtype=np.float32)
    g = np.arange(NG, dtype=np.float32)
    kki = (np.arange(KK) // K).astype(np.float32)
    kkj = (np.arange(KK) % K).astype(np.float32)
    # basex[p, g, kk] = p - 1 + kj
    basex = np.broadcast_to(
        (p[:, None, None] - 1.0 + kkj[None, None, :]) + 0.0 * g[None, :, None],
        (128, NG, KK))

    in_maps = []
    for core in range(8):
        b, h0 = core // 2, (core % 2) * HL
        by = np.broadcast_to(
            (h0 + g)[None, :, None] - 1.0 + kki[None, None, :],
            (128, NG, KK))
        byx = np.empty((128, NG, KK, 2), np.float32)
        byx[..., 0] = by
        byx[..., 1] = basex
        in_maps.append({
            "xp": np.ascontiguousarray(
                xp_full[b, :, h0:h0 + H2, :]).reshape(
                    C, H2 * W2).astype(_BF),
            "xcl": np.ascontiguousarray(xcl_full[b]).reshape(H4 * W4, 4 * C),
            "wofft": wofft, "woffb": woffb,
            "wmain": wmain, "wb": wb,
            "baseyx": byx.reshape(128, NG * KK * 2),
            "identb": identb, "selr": selr,
        })
    return in_maps


def kernel(x, weight, bias, offset_w, offset_b):
    nc = _get_nc()
    in_maps = make_core_inputs(x, weight, bias, offset_w, offset_b)
    res = run_bass_kernel_spmd(nc, in_maps, list(range(8)))
    out_full = np.empty((B, O, H, W), np.float32)
    for core in range(8):
        b, h0 = core // 2, (core % 2) * HL
        out_full[b, :, h0:h0 + HL, :] = res.results[core]["out"].reshape(O, HL, W)
    return out_full


# revision 24
# speedup vs baseline: 1.1205x; 1.1205x over previous
"""Deformable Conv2d (K=3, stride 1, pad 1, dil 1) on 8 TRN2 NeuronCores.

Sharding: data-parallel over (batch=4) x (H halves=2) -> 8 cores.
Each core computes out[b, :, h0:h0+64, :] for its (b, h0).

v3 pipeline (dma_gather + bf16):
  1. offset conv (18ch) via PE f32 matmuls over a 1px-zero-padded image.
  2. PE-transpose offsets to point-major [128pts, 18].
  3. DVE coord math (f32): ys/xs, magic-number floor, frac (stored bf16),
     bilinear corner weights w00..w11 (bf16), int16 gather row indices
     into a 2px-zero-padded channels-last 4C-packed bf16 image in DRAM.
  4. idx shuffle to the SWDGE dma_gather wrapped layout (idx j read at
     partition 16 + j%16, halfword j//16 -- HW-probed): hop1 = 8 small
     SBUF->SBUF DMAs moving partition blocks, hop2 = 3 DVE int16 copies
     permuting free dims.
  5. Gather: 3x dma_gather (InstDMAGatherAnt, mlp gpsimd library) per
     2-group tile, NI=768 rows x 512B (2x2 px x 64ch bf16, 4C-packed).
     One Pool instruction generates 768 descriptors (vs 6 instructions
     of 128 each for indirect_dma_start); ucode ring caps NI at 1024.
  6. DVE bilinear lerp in bf16: s = w00*v00+w01*v01+w10*v10+w11*v11
     (7 wide ops per group).
  7. PE transpose S (bf16) to channel-major, main conv matmuls
     (K=576 as 4x128+64 accumulation, bf16), ACT bias add (f32), DMA out.
"""

import sys
for p in ("/opt/trn_rl_repo",):
    if p not in sys.path:
        sys.path.insert(0, p)

import numpy as np
import ml_dtypes

_BF = ml_dtypes.bfloat16

import concourse.bacc as bacc
import concourse.mybir as mybir
import concourse.tile as tile
import concourse.bass as bass
from concourse.bass_utils import run_bass_kernel_spmd
from concourse.library_config import mlp as _mlp_lib

F32 = mybir.dt.float32
F32R = mybir.dt.float32r
BF16 = mybir.dt.bfloat16
I16 = mybir.dt.int16
AL = mybir.AluOpType
AF = mybir.ActivationFunctionType

B, C, H, W = 4, 64, 128, 128
K, KK = 3, 9
O = 64                      # output channels
OC = 2 * KK                 # offset channels (18)
HL = H // 2                 # local rows per core (64)
NPT = HL * W                # local points per core (8192)
NG = NPT // 128             # point groups of 128 (=64); group g == local row g
W2 = W + 2                  # 1px-padded width for offset conv (130)
H2 = HL + 2                 # 1px-padded local rows (66)
W4 = W + 4                  # 2px-padded width for gather image (132)
H4 = H + 4                  # 2px-padded height (full image!) (132)
MAGIC = float(3 * 2 ** 22)   # 1.5*2^23: ulp stays 1.0 for f32 in [-2^22, 2^22]
GCH = 2                     # point-groups per gather tile
E = 256                     # gathered elems per (point, tap): 2x2 px x 64ch
GBLK = KK * E               # gathered elems per point per group (2304)
NB = GCH * KK               # blocks per gather tile (18)
NCALL = 3                   # dma_gather calls per tile (6 blocks each)
NI = (NB // NCALL) * 128    # idxs per dma_gather call (768; ucode cap 1024)
CHUNKS = 8                  # preamble chunks (groups per chunk = NG/CHUNKS)
CG = NG // CHUNKS           # groups per chunk (16)
NT = CG // GCH              # gather tiles per chunk (8)
WCOL = CG * KK * 8          # wrapped idx cols per chunk (1152)


def build_program(dbg=False, skip_gather=False, skip_lerp=False,
                  skip_mm=False, skip_off=False, reps=1):
    nc = bacc.Bacc("TRN2", target_bir_lowering=False, debug=False)

    xp = nc.dram_tensor("xp", [C, H2 * W2], BF16, kind="ExternalInput")
    xcl = nc.dram_tensor("xcl", [H4 * W4, 4 * C], BF16, kind="ExternalInput")
    wofft = nc.dram_tensor("wofft", [C, KK * OC], BF16, kind="ExternalInput")
    woffb = nc.dram_tensor("woffb", [OC, 1], F32, kind="ExternalInput")
    wmain = nc.dram_tensor("wmain", [128, 5 * O], BF16, kind="ExternalInput")
    wb = nc.dram_tensor("wb", [O, 1], F32, kind="ExternalInput")
    baseyx = nc.dram_tensor("baseyx", [128, NG * KK * 2], F32,
                            kind="ExternalInput")
    identb = nc.dram_tensor("identb", [128, 128], BF16, kind="ExternalInput")
    selr = nc.dram_tensor("selr", [128, 8 * 32], F32, kind="ExternalInput")
    out = nc.dram_tensor("out", [O, NPT], F32, kind="ExternalOutput")

    # before TileContext: guaranteed to precede every gather (the tile
    # scheduler is free to move dep-less instructions otherwise, and a
    # GPSIMD iram reload between inflight dma_gathers is fatal).
    nc.gpsimd.load_library(_mlp_lib)

    with tile.TileContext(nc) as tc:
        with (
            tc.tile_pool(name="cst", bufs=1) as cst,
            tc.tile_pool(name="psA", bufs=1, space="PSUM") as psA,
            tc.tile_pool(name="psT", bufs=3, space="PSUM") as psT,
            tc.tile_pool(name="psO", bufs=3, space="PSUM") as psO,
        ):
            # ---- load constants / weights (once) ----
            identb_t = cst.tile([128, 128], BF16, tag="identb")
            nc.sync.dma_start(out=identb_t[:], in_=identb[:])
            selr_t = cst.tile([128, 8 * 32], F32, tag="selr")
            nc.sync.dma_start(out=selr_t[:], in_=selr[:])
            wofft_t = cst.tile([C, KK * OC], BF16, tag="wofft")
            nc.sync.dma_start(out=wofft_t[:], in_=wofft[:])
            woffb_t = cst.tile([OC, 1], F32, tag="woffb")
            nc.sync.dma_start(out=woffb_t[:], in_=woffb[:])
            wmain_t = cst.tile([128, 5 * O], BF16, tag="wmain")
            nc.sync.dma_start(out=wmain_t[:], in_=wmain[:])
            wb_t = cst.tile([O, 1], F32, tag="wb")
            nc.sync.dma_start(out=wb_t[:], in_=wb[:])
            baseyx_t = cst.tile([128, NG * KK * 2], F32, tag="baseyx")
            nc.sync.dma_start(out=baseyx_t[:], in_=baseyx[:])

            for rep in range(reps):
                build_body(nc, tc, psA, psT, psO, rep,
                           selr_t, identb_t, wofft_t, woffb_t, wmain_t,
                           wb_t, baseyx_t, xp, xcl, out,
                           skip_gather, skip_lerp, skip_mm, skip_off)

    nc.compile()
    return nc


def build_body(nc, tc, psA, psT, psO, rep,
               selr_t, identb_t, wofft_t, woffb_t, wmain_t, wb_t,
               baseyx_t, xp, xcl, out,
               skip_gather, skip_lerp, skip_mm, skip_off):
    with (
        tc.tile_pool(name=f"early{rep}", bufs=1) as early,
        tc.tile_pool(name=f"coord{rep}", bufs=1) as coord,
        tc.tile_pool(name=f"ctmp{rep}", bufs=2) as ctmp,
        tc.tile_pool(name=f"gat{rep}", bufs=3) as gat,
        tc.tile_pool(name=f"lrp{rep}", bufs=2) as lrp,
        tc.tile_pool(name=f"outp{rep}", bufs=1) as outp,
    ):
        xp_t = early.tile([C, H2 * W2], BF16, tag="xp")
        nc.sync.dma_start(out=xp_t[:], in_=xp[:])
        xp3 = xp_t[:].rearrange("c (h w) -> c h w", h=H2)

        # per-chunk coord tiles so chunk-0 gathers don't wait on chunk-3
        w_c = []      # per chunk: [w00, w01, w10, w11] bf16 [128, CG*KK]
        wrap_c = []   # per chunk: wrapped int16 idx [128, WCOL]
        for ch in range(CHUNKS):
            w_c.append([coord.tile([128, CG * KK], BF16, tag=f"w{q}{ch}",
                                   name=f"w{q}{ch}_{rep}") for q in range(4)])
            wrap_c.append(coord.tile([128, WCOL], I16, tag=f"wr{ch}",
                                     name=f"wr{ch}_{rep}"))
        off_c = [coord.tile([OC, CG * W], BF16, tag=f"off{ch}",
                            name=f"off{ch}_{rep}")
                 for ch in range(CHUNKS)]
        out_sb = outp.tile([O, NPT], F32, tag="osb")
        if skip_mm:
            nc.vector.memset(out_sb[:], 0.0)

        def preamble_chunk(ch):
            g0 = ch * CG          # first group (= local row) of chunk
            off_t = off_c[ch]
            if skip_off:
                nc.vector.memset(off_t[:], 0.0)
            # ---- offset conv rows [g0, g0+CG) ----
            RPC = 4               # rows per psum chunk (N=512)
            for r0 in ([] if skip_off else range(g0, g0 + CG, RPC)):
                ps = psA.tile([OC, RPC * W], F32, tag="psA")
                for kk in range(KK):
                    ki, kj = kk // K, kk % K
                    rhs = xp3[:, r0 + ki:r0 + ki + RPC, kj:kj + W]
                    nc.tensor.matmul(
                        out=ps[:],
                        lhsT=wofft_t[:, kk * OC:(kk + 1) * OC],
                        rhs=rhs,
                        start=(kk == 0), stop=(kk == KK - 1))
                nc.scalar.activation(
                    out=off_t[:, (r0 - g0) * W:(r0 - g0 + RPC) * W], in_=ps[:],
                    func=AF.Identity, bias=woffb_t[:, 0:1], scale=1.0)

            # ---- transpose offsets to point-major [128, CG*18] ----
            offT = ctmp.tile([128, CG * OC], BF16, tag="offT")
            for gl in range(CG):
                ps = psT.tile([128, OC], BF16, tag="psTb")
                nc.tensor.transpose(
                    out=ps[:], in_=off_t[:, gl * 128:(gl + 1) * 128],
                    identity=identb_t[:OC, :OC])
                nc.scalar.copy(out=offT[:, gl * OC:(gl + 1) * OC], in_=ps[:])

            # ---- coordinate math on the (g,k,(y|x))-interleaved layout
            # offT is already (dy,dx)-interleaved, so ys|xs, floor, frac,
            # 1-frac and clamp all run as single double-width ops.
            NW = CG * KK
            yx = ctmp.tile([128, 2 * NW], F32, tag="yx")
            rr = ctmp.tile([128, 2 * NW], F32, tag="rr")
            mm_ = ctmp.tile([128, 2 * NW], F32, tag="mm")
            yx0 = ctmp.tile([128, 2 * NW], F32, tag="yx0")
            fyx = ctmp.tile([128, 2 * NW], BF16, tag="fyx")
            gyx = ctmp.tile([128, 2 * NW], BF16, tag="gyx")
            ti = ctmp.tile([128, NW], F32, tag="ti")
            idxf = ctmp.tile([128, NW], F32, tag="idxf")

            nc.vector.tensor_tensor(
                out=yx[:], in0=offT[:],
                in1=baseyx_t[:, g0 * 2 * KK:(g0 + CG) * 2 * KK], op=AL.add)
            # magic-number round-to-nearest, then fix round-ups
            nc.vector.tensor_scalar(
                out=rr[:], in0=yx[:], scalar1=MAGIC, scalar2=MAGIC,
                op0=AL.add, op1=AL.subtract)
            nc.vector.tensor_tensor(out=mm_[:], in0=rr[:], in1=yx[:],
                                    op=AL.is_gt)
            nc.vector.tensor_tensor(out=yx0[:], in0=rr[:], in1=mm_[:],
                                    op=AL.subtract)
            nc.vector.tensor_tensor(out=fyx[:], in0=yx[:], in1=yx0[:],
                                    op=AL.subtract)
            # gyx = (fyx - 1) * -1 = 1 - fyx
            nc.vector.tensor_scalar(out=gyx[:], in0=fyx[:], scalar1=1.0,
                                    scalar2=-1.0, op0=AL.subtract,
                                    op1=AL.mult)
            # corner weights from interleaved frac views
            f4 = fyx[:].rearrange("p (g k t) -> p g k t", g=CG, k=KK)
            g4 = gyx[:].rearrange("p (g k t) -> p g k t", g=CG, k=KK)
            fy_v, fx_v = f4[:, :, :, 0], f4[:, :, :, 1]
            gy_v, gx_v = g4[:, :, :, 0], g4[:, :, :, 1]
            w00, w01, w10, w11 = w_c[ch]
            w003 = w00[:].rearrange("p (g k) -> p g k", g=CG)
            w013 = w01[:].rearrange("p (g k) -> p g k", g=CG)
            w103 = w10[:].rearrange("p (g k) -> p g k", g=CG)
            w113 = w11[:].rearrange("p (g k) -> p g k", g=CG)
            nc.vector.tensor_tensor(out=w003, in0=gy_v, in1=gx_v, op=AL.mult)
            nc.vector.tensor_tensor(out=w013, in0=gy_v, in1=fx_v, op=AL.mult)
            nc.vector.tensor_tensor(out=w103, in0=fy_v, in1=gx_v, op=AL.mult)
            nc.vector.tensor_tensor(out=w113, in0=fy_v, in1=fx_v, op=AL.mult)
            # clamp both coords (same bounds: [-2, 128] since H == W)
            nc.vector.tensor_scalar(out=rr[:], in0=yx0[:], scalar1=-2.0,
                                    scalar2=float(H), op0=AL.max, op1=AL.min)
            # idx = (y0c*W4 + x0c) + (2*W4+2), f32 (exact integers)
            r4 = rr[:].rearrange("p (g k t) -> p g k t", g=CG, k=KK)
            ti3 = ti[:].rearrange("p (g k) -> p g k", g=CG)
            nc.vector.scalar_tensor_tensor(
                out=ti3, in0=r4[:, :, :, 0], scalar=float(W4),
                in1=r4[:, :, :, 1], op0=AL.mult, op1=AL.add)
            nc.vector.tensor_scalar(
                out=idxf[:], in0=ti[:], scalar1=float(2 * W4 + 2),
                scalar2=None, op0=AL.add)

            # ---- shuffle idxf [128 pts, NW] -> SWDGE wrapped layout ----
            # Gather ucode reads idx j at (partition 16 + j%16,
            # halfword j//16) of the idxs AP (HW-probed).  Target cell for
            # idx j of (tile t, call c, block b, point p=16*r+q):
            # (16+q, t*144 + c*48 + b*8 + r).  All-compute path (PE fold +
            # DVE strided int16 write) so the gather depends only on
            # engine semaphores, not rotating DMA-completion counters.
            ovw = wrap_c[ch][0:32, :].rearrange(
                "p (t c b r) -> p t c b r", t=NT, c=NCALL, b=NB // NCALL)
            for r in range(8):
                # psW[16+q, col] = idxf[16*r+q, col]; rows 0..15 zero
                ps = psA.tile([32, NW], F32, tag="psW")
                nc.tensor.matmul(
                    out=ps[:], lhsT=selr_t[:, r * 32:(r + 1) * 32],
                    rhs=idxf[:], start=True, stop=True)
                # f32 -> int16 with the (t,c,b) -> stride-8 col scatter
                nc.vector.tensor_scalar(
                    out=ovw[:, :, :, :, r], in0=ps[:].rearrange(
                        "p (t c b) -> p t c b", t=NT, c=NCALL),
                    scalar1=0, scalar2=None, op0=AL.add)

        def main_groups(ch):
            g0 = ch * CG
            for c0 in range(g0, g0 + CG, GCH):
                t = (c0 - g0) // GCH      # tile index within chunk
                gt = gat.tile([128, GCH * GBLK], BF16, tag="G")
                if skip_gather and not skip_lerp:
                    nc.vector.memset(gt[:], 0.0)
                # 3 dma_gather calls of 6 blocks (768 idxs) each: one Pool
                # instruction generates 768 descriptors of 512B.  NOTE:
                # multi-queue round-robin (num_swdge_queues=2) hangs the
                # mesh in this kernel -- single queue only.
                for c in ([] if skip_gather else range(NCALL)):
                    nc.gpsimd.dma_gather(
                        out_ap=gt[:, c * 6 * E:(c + 1) * 6 * E].rearrange(
                            "p (g e) -> p g e", e=E),
                        in_ap=xcl[:],
                        idxs_ap=wrap_c[ch][:, t * 144 + c * 48:
                                           t * 144 + (c + 1) * 48],
                        num_idxs=NI, num_idxs_reg=NI, elem_size=E,
                        queue_num=0)
                for gs in range(GCH):
                    g = c0 + gs
                    gl = g - g0
                    g5 = gt[:, gs * GBLK:(gs + 1) * GBLK].rearrange(
                        "p (k r q c) -> p k r q c", k=KK, r=2, q=2)
                    v00 = g5[:, :, 0, 0, :]
                    v01 = g5[:, :, 0, 1, :]
                    v10 = g5[:, :, 1, 0, :]
                    v11 = g5[:, :, 1, 1, :]

                    def wb_(wt):
                        return wt[:, gl * KK:(gl + 1) * KK].unsqueeze(2) \
                            .to_broadcast([128, KK, C])

                    d_ = lrp.tile([128, KK * C], BF16, tag="d")
                    m_ = lrp.tile([128, KK * C], BF16, tag="m")
                    l0 = lrp.tile([128, KK * C], BF16, tag="l0")
                    l1 = lrp.tile([128, KK * C], BF16, tag="l1")
                    sg = lrp.tile([128, KK * C], BF16, tag="s")
                    if skip_lerp and not skip_mm:
                        nc.vector.memset(sg[:], 0.0)
                    d3 = d_[:].rearrange("p (k c) -> p k c", k=KK)
                    m3 = m_[:].rearrange("p (k c) -> p k c", k=KK)
                    l03 = l0[:].rearrange("p (k c) -> p k c", k=KK)
                    l13 = l1[:].rearrange("p (k c) -> p k c", k=KK)
                    s3 = sg[:].rearrange("p (k c) -> p k c", k=KK)

                    if not skip_lerp:
                        w00, w01, w10, w11 = w_c[ch]
                        nc.vector.tensor_tensor(out=d3, in0=v00,
                                                in1=wb_(w00), op=AL.mult)
                        nc.vector.tensor_tensor(out=m3, in0=v01,
                                                in1=wb_(w01), op=AL.mult)
                        nc.vector.tensor_tensor(out=l03, in0=d3, in1=m3,
                                                op=AL.add)
                        nc.vector.tensor_tensor(out=d3, in0=v10,
                                                in1=wb_(w10), op=AL.mult)
                        nc.vector.tensor_tensor(out=m3, in0=v11,
                                                in1=wb_(w11), op=AL.mult)
                        nc.vector.tensor_tensor(out=l13, in0=d3, in1=m3,
                                                op=AL.add)
                        nc.vector.tensor_tensor(out=s3, in0=l03, in1=l13,
                                                op=AL.add)

                    if skip_mm:
                        continue
                    st = lrp.tile([128, 640], BF16, tag="st")
                    for j in range(4):
                        ps = psT.tile([128, 128], BF16, tag="psTb")
                        nc.tensor.transpose(
                            out=ps[:], in_=sg[:, j * 128:(j + 1) * 128],
                            identity=identb_t[:])
                        if j % 2 == 0:
                            nc.scalar.copy(out=st[:, j * 128:(j + 1) * 128],
                                           in_=ps[:])
                        else:
                            # balance PSUM->SBUF copies across ACT and DVE
                            nc.vector.tensor_scalar(
                                out=st[:, j * 128:(j + 1) * 128], in0=ps[:],
                                scalar1=0.0, scalar2=None, op0=AL.add)
                    ps = psT.tile([64, 128], BF16, tag="psTb")
                    nc.tensor.transpose(
                        out=ps[:], in_=sg[:, 512:576], identity=identb_t[:])
                    nc.scalar.copy(out=st[:64, 512:640], in_=ps[:])

                    po = psO.tile([O, 128], F32, tag="psO")
                    for j in range(4):
                        nc.tensor.matmul(
                            out=po[:], lhsT=wmain_t[:, j * O:(j + 1) * O],
                            rhs=st[:, j * 128:(j + 1) * 128],
                            start=(j == 0), stop=False)
                    nc.tensor.matmul(
                        out=po[:], lhsT=wmain_t[:64, 4 * O:5 * O],
                        rhs=st[:64, 512:640], start=False, stop=True)
                    nc.scalar.activation(
                        out=out_sb[:, g * 128:(g + 1) * 128], in_=po[:],
                        func=AF.Identity, bias=wb_t[:, 0:1], scale=1.0)
            # out DMA per chunk (overlaps with next chunk's work)
            nc.sync.dma_start(
                out=out[:, g0 * 128:(g0 + CG) * 128],
                in_=out_sb[:, g0 * 128:(g0 + CG) * 128])

        # chunk 0 preamble, then stream: gathers(ch) while preamble(ch+1)
        preamble_chunk(0)
        for ch in range(CHUNKS):
            if ch + 1 < CHUNKS:
                preamble_chunk(ch + 1)
            main_groups(ch)


_NC_CACHE = None


def _get_nc():
    global _NC_CACHE
    if _NC_CACHE is None:
        _NC_CACHE = build_program()
    return _NC_CACHE


def make_core_inputs(x, weight, bias, offset_w, offset_b):
    """Host-side prep: returns list of 8 in_maps (core i = batch i//2, half i%2)."""
    x = np.asarray(x, np.float32)
    weight = np.asarray(weight, np.float32)
    bias = np.asarray(bias, np.float32)
    offset_w = np.asarray(offset_w, np.float32)
    offset_b = np.asarray(offset_b, np.float32)

    xp_full = np.pad(x, ((0, 0), (0, 0), (1, 1), (1, 1)))
    xpad = np.pad(x, ((0, 0), (0, 0), (2, 2), (2, 3)))  # extra right col for x+1
    xpad = np.pad(xpad, ((0, 0), (0, 0), (0, 1), (0, 0)))  # extra bottom row
    xcl0 = xpad.transpose(0, 2, 3, 1)           # [B, 133, 133, C]
    zz = np.empty((B, H4, W4, 4 * C), np.float32)
    zz[..., 0 * C:1 * C] = xcl0[:, :H4, :W4, :]
    zz[..., 1 * C:2 * C] = xcl0[:, :H4, 1:W4 + 1, :]
    zz[..., 2 * C:3 * C] = xcl0[:, 1:H4 + 1, :W4, :]
    zz[..., 3 * C:4 * C] = xcl0[:, 1:H4 + 1, 1:W4 + 1, :]
    xcl_full = zz.astype(_BF)

    # offset conv weights: [c, kk*18], lhsT per tap
    wofft = np.ascontiguousarray(
        offset_w.reshape(OC, C, KK).transpose(1, 2, 0)).reshape(
            C, KK * OC).astype(_BF)
    woffb = offset_b.reshape(OC, 1)
    # main conv weights: [128, 5*64]; block j rows (t2*64+c), cols o
    wr = weight.reshape(O, C, KK)
    wmain = np.zeros((128, 5 * O), np.float32)
    for j in range(5):
        for t2 in range(2):
            kk = 2 * j + t2
            if kk >= KK:
                break
            wmain[t2 * C:(t2 + 1) * C, j * O:(j + 1) * O] = wr[:, :, kk].T
    wmain = wmain.astype(_BF)
    wb = bias.reshape(O, 1)
    identb = np.eye(128, dtype=np.float32).astype(_BF)
    # selr[p, r*32 + c] = 1 iff c >= 16 and p == 16*r + (c - 16):
    # fold matmul lhsT placing point 16*r+q at psum row 16+q
    selr = np.zeros((128, 8 * 32), np.float32)
    for r in range(8):
        for q in range(16):
            selr[16 * r + q, r * 32 + 16 + q] = 1.0

    p = np.arange(128, dtype=np.float32)
    g = np.arange(NG, dtype=np.float32)
    kki = (np.arange(KK) // K).astype(np.float32)
    kkj = (np.arange(KK) % K).astype(np.float32)
    # basex[p, g, kk] = p - 1 + kj
    basex = np.broadcast_to(
        (p[:, None, None] - 1.0 + kkj[None, None, :]) + 0.0 * g[None, :, None],
        (128, NG, KK))

    in_maps = []
    for core in range(8):
        b, h0 = core // 2, (core % 2) * HL
        by = np.broadcast_to(
            (h0 + g)[None, :, None] - 1.0 + kki[None, None, :],
            (128, NG, KK))
        byx = np.empty((128, NG, KK, 2), np.float32)
        byx[..., 0] = by
        byx[..., 1] = basex
        in_maps.append({
            "xp": np.ascontiguousarray(
                xp_full[b, :, h0:h0 + H2, :]).reshape(
                    C, H2 * W2).astype(_BF),
            "xcl": np.ascontiguousarray(xcl_full[b]).reshape(H4 * W4, 4 * C),
            "wofft": wofft, "woffb": woffb,
            "wmain": wmain, "wb": wb,
            "baseyx": byx.reshape(128, NG * KK * 2),
            "identb": identb, "selr": selr,
        })
    return in_maps


def kernel(x, weight, bias, offset_w, offset_b):
    nc = _get_nc()
    in_maps = make_core_inputs(x, weight, bias, offset_w, offset_b)
    res = run_bass_kernel_spmd(nc, in_maps, list(range(8)))
    out_full = np.empty((B, O, H, W), np.float32)
    for core in range(8):
        b, h0 = core // 2, (core % 2) * HL
        out_full[b, :, h0:h0 + HL, :] = res.results[core]["out"].reshape(O, HL, W)
    return out_full
